# revision 1
# baseline (speedup 1.0000x reference)
"""Trainium2 Bass kernel for nn_HOR_16870631539538 (dense_transformer).

Module (per batch item b, C=64 channels, hw=4096 spatial):
  stage 1: p = x_low^T conv outputs attention [hw,hw], softmax over axis n,
           e = p_sm @ v + x_low
  stage 2: t = conv_e(e) @ xl2_sp  (64x64), softmax over c, out = x_mid @ t_sm

Sharding: 8 cores = 4 batch items x 2 halves of the softmax-column dim (m).
Each core computes exp/softmax for its m-half only (the expensive part).
Key algebraic trick: downstream only needs G = e @ xl2_sp  (64x65 incl. the
ones-row for the e_conv bias term), which is linear in the m-partial e, so the
cross-core combine is ONE 16KB AllReduce of G instead of 1MiB of e.

Layouts: conv outputs channel-major [c, n]; attention computed transposed
pT[m, n] so softmax axis n is the free dim (ACT exp with fused accum_out).
The 1/denominator is folded into the small v matrix, never touching the slab.

Dtypes: attention matmuls in fp16 (11-bit, inputs O(10)); exp slab + vs in
bf16 (range-safe, no max-subtraction needed: |p| < ~60); everything feeding
stage-2 t in fp32/fp16 where exact/cheap.
"""

import numpy as np

import concourse.bacc as bacc
import concourse.bass as bass
import concourse.mybir as mybir
import concourse.tile as tile
from concourse.bass_utils import run_bass_kernel_spmd

dt = mybir.dt
AF = mybir.ActivationFunctionType
ALU = mybir.AluOpType

N_CORES = 8
C = 64
HW = 4096
MH = HW // 2           # per-core m-half (2048)
NCHUNK = MH // 128     # 16 m-chunks of 128 rows
NB = HW // 512         # 8 n-blocks of 512

# dtypes for the two big matmul stages
DT_PT = dt.float32r    # pT matmul operands (xl_hi, xlowT)
DT_SLAB = dt.bfloat16  # exp slab + vs (bf16: range-safe for exp w/o max)
DT_G = dt.float32      # e_sp / xl2sp operands for the G matmuls
DT_OUT = dt.float32r   # tsm / xmidT operands for the output matmul

USE_COLLECTIVE = True
TRUNC = 99   # debug: 1=convs 2=+transposes 3=+mloop 4=+G 99=full

_CACHE = {}


def build():
    nc = bacc.Bacc("TRN2", target_bir_lowering=False, debug=False,
                   num_devices=N_CORES)

    def din(name, shape, dtype=dt.float32):
        return nc.dram_tensor(name, shape, dtype, kind="ExternalInput").ap()

    xin = din("xin", [C, HW], dt.float32r)      # x[b] channel-major
    xlat = din("xlat", [C, HW], dt.float32r)    # x_latter[b]
    xlat_mh = din("xlat_mh", [C, MH], dt.float32r)
    x_mh = din("x_mh", [C, MH], dt.float32r)
    wlT = din("wlT", [C, C], dt.float32r)
    whT = din("whT", [C, C], dt.float32r)
    wvT = din("wvT", [C, C], dt.float32r)
    weT = din("weT", [C, C])
    wlatT = din("wlatT", [C, C], dt.float32r)
    wmT = din("wmT", [C, C], dt.float32r)
    bl = din("bl", [C, 1])
    bh = din("bh", [C, 1])
    bv = din("bv", [C, 1])
    blat = din("blat", [C, 1])
    bm = din("bm", [C, 1])
    be1 = din("be1", [1, C])           # e_conv bias as a row
    idf32 = din("idf32", [C, C])       # identity fp32
    id16 = nc.dram_tensor("id16", [128, C], dt.float16,
                          kind="ExternalInput").ap()   # stacked identity fp16
    halfid = nc.dram_tensor("halfid", [C, C], dt.float32r,
                            kind="ExternalInput").ap()  # 0.5*I
    outp = nc.dram_tensor("outp", [C, MH], dt.float32,
                          kind="ExternalOutput").ap()

    with tile.TileContext(nc) as tc:
        _body(nc, tc, locals())
    nc.compile()
    return nc


def _body(nc, tc, io):
    ts = bass.ts

    const = tc.alloc_tile_pool(name="const", bufs=1)
    big = tc.alloc_tile_pool(name="big", bufs=1)
    slabp = tc.alloc_tile_pool(name="slabp", bufs=2)
    mm = tc.alloc_tile_pool(name="mm", bufs=2, space="PSUM")
    acc = tc.alloc_tile_pool(name="acc", bufs=1, space="PSUM")
    dram = tc.alloc_tile_pool(name="dram", bufs=1, space="DRAM")

    # ---- load constants ----
    def cload(name, shape, dtype=dt.float32):
        t = const.tile(shape, dtype, tag=name)
        nc.sync.dma_start(t[:], io[name])
        return t

    wlT = cload("wlT", [C, C], dt.float32r)
    whT = cload("whT", [C, C], dt.float32r)
    wvT = cload("wvT", [C, C], dt.float32r)
    weT = cload("weT", [C, C])
    wlatT = cload("wlatT", [C, C], dt.float32r)
    wmT = cload("wmT", [C, C], dt.float32r)
    bl = cload("bl", [C, 1]); bh = cload("bh", [C, 1])
    bv = cload("bv", [C, 1]); blat = cload("blat", [C, 1])
    bm = cload("bm", [C, 1])
    idf32 = cload("idf32", [C, C])
    id16 = cload("id16", [128, C], dt.float16)
    halfid = cload("halfid", [C, C], dt.float32r)
    be_t = const.tile([65, C], dt.float32, tag="be_t")
    nc.sync.dma_start(be_t[64:65, :], io["be1"])

    # ---- load inputs ----
    xin = big.tile([C, HW], dt.float32r, tag="xin")
    xlat = big.tile([C, HW], dt.float32r, tag="xlat")
    xlat_mh = big.tile([C, MH], dt.float32r, tag="xlat_mh")
    x_mh = big.tile([C, MH], dt.float32r, tag="x_mh")
    nc.sync.dma_start(xin[:], io["xin"])
    nc.sync.dma_start(xlat[:], io["xlat"])
    nc.sync.dma_start(xlat_mh[:], io["xlat_mh"])
    nc.sync.dma_start(x_mh[:], io["x_mh"])

    # ---- conv helper: out_sbuf[c, n] = W @ x + b, evicted via DVE ----
    def conv(dst, wT, src, bias, ncols):
        for j in range(0, ncols, 1024):
            w = min(1024, ncols - j)
            pt = mm.tile([C, 1024], dt.float32, tag="mmt")
            for k in range(0, w, 512):
                nc.tensor.matmul(pt[:, k:k + min(512, w - k)], wT[:],
                                 src[:, j + k:j + k + min(512, w - k)],
                                 start=True, stop=True)
            nc.vector.tensor_scalar(dst[:, j:j + w], pt[:, 0:w], bias[:], None,
                                    ALU.add)

    def conv_accum(dst, wT, src, bias, ncols, accum):
        # like conv but also per-partition row-sum accumulation parts
        nparts = (ncols + 1023) // 1024
        for ji, j in enumerate(range(0, ncols, 1024)):
            w = min(1024, ncols - j)
            pt = mm.tile([C, 1024], dt.float32, tag="mmt")
            for k in range(0, w, 512):
                nc.tensor.matmul(pt[:, k:k + min(512, w - k)], wT[:],
                                 src[:, j + k:j + k + min(512, w - k)],
                                 start=True, stop=True)
            nc.vector.tensor_scalar(dst[:, j:j + w], pt[:, 0:w], bias[:], 0.0,
                                    ALU.add, ALU.add,
                                    accum_out=accum[:, ji:ji + 1])
        return nparts

    xlowT = big.tile([C, HW], DT_PT, tag="xlowT")      # conv_low(x), full n
    xl_hi = big.tile([C, MH], DT_PT, tag="xl_hi")      # conv_high(xlat) m-half
    v_s = big.tile([C, MH], dt.float32, tag="v_s")     # conv_value(x) m-half
    xl2 = big.tile([C, HW], DT_G, tag="xl2")           # conv_latter(xlat) full
    xmidT = big.tile([C, MH], DT_OUT, tag="xmidT")     # conv_mid(x) n-half
    sacc = big.tile([C, 4], dt.float32, tag="sacc")    # xl2 row-sum parts

    conv(xlowT, wlT, xin, bl, HW)
    conv(xl_hi, whT, xlat_mh, bh, MH)
    conv(v_s, wvT, x_mh, bv, MH)
    conv_accum(xl2, wlatT, xlat, blat, HW, sacc)
    conv(xmidT, wmT, x_mh, bm, MH)

    # s[d] = sum_n xl2[d, n]  (per-partition scalar)
    s_col = big.tile([C, 1], dt.float32, tag="s_col")
    nc.vector.reduce_sum(s_col[:], sacc[:], axis=mybir.AxisListType.X)

    def finish(src_ap):
        osb = big.tile([C, MH], dt.float32, tag="osb")
        nc.gpsimd.memset(osb[:], 0.0)
        nc.vector.tensor_copy(osb[:, 0:src_ap.shape[1]], src_ap)
        nc.sync.dma_start(io["outp"], osb[:])
        for p in (dram, acc, mm, slabp, big, const):
            p.release()

    if TRUNC == 1:
        return finish(xlowT[:, 0:MH])

    # ---- transpose v: [64, MH] -> 16 tiles [128, 64] in v_sp ----
    v_sp = big.tile([128, NCHUNK * C], dt.float32, tag="v_sp")
    for g in range(0, NCHUNK, 8):  # batches of 8 transposes -> one psum tile
        pt = mm.tile([128, 512], dt.float32, tag="mmt")
        for q in range(8):
            i = g + q
            nc.tensor.transpose(pt[:, q * 64:(q + 1) * 64],
                                v_s[:, ts(i, 128)], idf32[:])
        nc.vector.tensor_copy(v_sp[:, g * 64:(g + 8) * 64], pt[:])

    # ---- transpose xl2: [64, HW] -> 32 tiles [128, 64] (fp16) ----
    xl2sp = big.tile([128, 32 * C], DT_G, tag="xl2sp")
    for g in range(0, 32, 8):
        pt = mm.tile([128, 512], DT_G, tag="mmt")
        for q in range(8):
            i = g + q
            nc.tensor.transpose(pt[:, q * 64:(q + 1) * 64],
                                xl2[:, ts(i, 128)], idf32[:])
        nc.vector.tensor_copy(xl2sp[:, g * 64:(g + 8) * 64], pt[:])

    if TRUNC == 2:
        return finish(xl2sp[0:64, 0:2048])

    # ---- eT accumulators: 4 psum tiles [128, 512]; partition half p holds
    # n-block 2k+p. Open each group with the 0.5*x_low residual matmul. ----
    accs = [acc.tile([128, 512], dt.float32, tag=f"acc{k}", name=f"acc{k}")
            for k in range(4)]

    # x_low residual in acc layout: partitions 0-63 = even n-blocks,
    # 64-127 = odd n-blocks (partition-moving SBUF DMA)
    xlow_acc = big.tile([128, 4 * 512], dt.float32, tag="xlow_acc")
    for k in range(4):
        nc.sync.dma_start(xlow_acc[0:64, ts(k, 512)],
                          xlowT[:, ts(2 * k, 512)].bitcast(dt.float32))
        nc.sync.dma_start(xlow_acc[64:128, ts(k, 512)],
                          xlowT[:, ts(2 * k + 1, 512)].bitcast(dt.float32))

    # ---- stage-1 m-loop ----
    for i in range(NCHUNK):
        slab = slabp.tile([128, HW], DT_SLAB, tag="slab")
        dacc = slabp.tile([128, 4], dt.float32, tag="dacc")
        for j in range(4):  # pT sub-passes of [128, 1024]
            pt = mm.tile([128, 1024], dt.float32, tag="mmt")
            for k in range(2):
                nc.tensor.matmul(pt[:, k * 512:(k + 1) * 512],
                                 xl_hi[:, ts(i, 128)],
                                 xlowT[:, j * 1024 + k * 512:
                                       j * 1024 + (k + 1) * 512],
                                 start=True, stop=True)
            nc.scalar.activation(slab[:, j * 1024:(j + 1) * 1024], pt[:],
                                 AF.Exp, accum_out=dacc[:, j:j + 1])
        dsum = slabp.tile([128, 1], dt.float32, tag="dsum")
        nc.vector.reduce_sum(dsum[:], dacc[:], axis=mybir.AxisListType.X)
        rec = slabp.tile([128, 1], dt.float32, tag="rec")
        nc.vector.reciprocal(rec[:], dsum[:])
        vs = slabp.tile([128, C], DT_SLAB, tag="vs")
        nc.vector.tensor_scalar(vs[:], v_sp[:, ts(i, C)], rec[:], None,
                                ALU.mult)
        first = i == 0
        last = i == NCHUNK - 1
        for k in range(4):
            for p in range(2):
                blk = 2 * k + p
                nc.tensor.matmul(accs[k][p * 64:(p + 1) * 64, :], vs[:],
                                 slab[:, ts(blk, 512)], start=first,
                                 stop=last, skip_group_check=True)

    # ---- evict e + 0.5*x_low residual (block-permuted [128, 2048]) ----
    e_h = big.tile([128, 4 * 512], DT_G, tag="e_h")
    for k in range(4):
        nc.vector.scalar_tensor_tensor(e_h[:, ts(k, 512)],
                                       xlow_acc[:, ts(k, 512)], 0.5,
                                       accs[k][:], ALU.mult, ALU.add)

    if TRUNC == 3:
        return finish(e_h[0:64, :])

    # move odd-block rows (partitions 64-127) down to 0-63 via SBUF DMA
    e_h2 = big.tile([C, 4 * 512], DT_G, tag="e_h2")
    nc.sync.dma_start(e_h2[:], e_h[64:128, :])

    # ---- transpose e -> e_sp tiles [128, 64] in n-tile order ----
    e_sp = big.tile([128, 32 * C], DT_G, tag="e_sp")
    for g in range(0, 32, 8):
        pt = mm.tile([128, 512], DT_G, tag="mmt")
        for q in range(8):
            t_idx = g + q              # n-tile index (n = t_idx*128)
            blk = t_idx // 4           # n-block
            sl = t_idx % 4             # 128-col slice within block
            k = blk // 2
            p = blk % 2
            src = e_h if p == 0 else e_h2
            nc.tensor.transpose(
                pt[:, q * 64:(q + 1) * 64],
                src[0:64, k * 512 + sl * 128:k * 512 + (sl + 1) * 128],
                idf32[:])
        nc.vector.tensor_copy(e_sp[:, g * 64:(g + 8) * 64], pt[:])

    if TRUNC == 35:
        return finish(e_sp[0:64, 0:2048])

    # ---- G = sum_n e_sp^T-pair xl2sp  -> [64(a), 64(d)] psum, fp16 mms ----
    gps = acc.tile([128, 512], dt.float32, tag="acc0")
    G = gps[0:64, 0:64]
    for t_idx in range(32):
        nc.tensor.matmul(G, e_sp[:, ts(t_idx, C)], xl2sp[:, ts(t_idx, C)],
                         start=(t_idx == 0), stop=(t_idx == 31),
                         skip_group_check=True)

    if TRUNC == 36:
        gtmp = big.tile([C, C], dt.float32, tag="gtmp")
        nc.vector.tensor_copy(gtmp[:], G)
        return finish(gtmp[:])

    # transpose s_col [64,1] -> s row [1, 64], scaled 0.5 at eviction
    spt = mm.tile([128, 512], dt.float32, tag="mmt")
    nc.tensor.transpose(spt[0:1, 0:64], s_col[:], idf32[:])

    gs_stage = big.tile([65, C], dt.float32, tag="gs_stage")
    nc.vector.tensor_copy(gs_stage[0:64, :], G)
    nc.vector.tensor_scalar(gs_stage[64:65, :], spt[0:1, 0:64], 0.5, None,
                            ALU.mult)

    if TRUNC == 4:
        return finish(gs_stage[0:64, :])

    # ---- AllReduce G over core pairs ----
    gs_red = big.tile([65, C], dt.float32, tag="gs_red")
    if USE_COLLECTIVE:
        g_in = dram.tile([65, C], dt.float32, tag="g_in")
        g_out = dram.tile([65, C], dt.float32, tag="g_out")
        nc.sync.dma_start(g_in[:], gs_stage[:])
        nc.gpsimd.collective_compute(
            "AllReduce", ALU.add,
            ins=[g_in.opt()], outs=[g_out.opt()],
            replica_groups=[[0, 1], [2, 3], [4, 5], [6, 7]],
        )
        nc.sync.dma_start(gs_red[:], g_out[:])
    else:
        nc.vector.tensor_copy(gs_red[:], gs_stage[:])

    # ---- t = We @ G + be x s  (fp32 matmuls) ----
    tps = mm.tile([128, 512], dt.float32, tag="mmt")
    t_ps = tps[0:64, 0:64]
    nc.tensor.matmul(t_ps, weT[:], gs_red[0:64, :], start=True, stop=False,
                     skip_group_check=True)
    nc.tensor.matmul(t_ps, be_t[64:65, :], gs_red[64:65, :], start=False,
                     stop=True, tile_position=(64, 0), skip_group_check=True)
    t_s = big.tile([C, C], dt.float32, tag="t_s")
    nc.vector.tensor_copy(t_s[:], t_ps)

    # ---- softmax over c: transpose -> [d, c], exp w/ max, normalize ----
    tt_ps = mm.tile([128, 512], dt.float32, tag="mmt")
    nc.tensor.transpose(tt_ps[0:64, 0:64], t_s[:], idf32[:])
    tmax = big.tile([C, 1], dt.float32, tag="tmax")
    nc.vector.reduce_max(tmax[:], tt_ps[0:64, 0:64], axis=mybir.AxisListType.X)
    nmax = big.tile([C, 1], dt.float32, tag="nmax")
    nc.vector.tensor_scalar(nmax[:], tmax[:], -1.0, None, ALU.mult)
    texp = big.tile([C, C], dt.float32, tag="texp")
    tsum = big.tile([C, 1], dt.float32, tag="tsum")
    nc.scalar.activation(texp[:], tt_ps[0:64, 0:64], AF.Exp, bias=nmax[:],
                         accum_out=tsum[:])
    trec = big.tile([C, 1], dt.float32, tag="trec")
    nc.vector.reciprocal(trec[:], tsum[:])
    tsmT = big.tile([C, C], dt.float32, tag="tsmT")
    nc.vector.tensor_scalar(tsmT[:], texp[:], trec[:], None, ALU.mult)

    # transpose back -> tsm[c, d] fp16 for the output matmul
    tb_ps = mm.tile([128, 512], dt.float32, tag="mmt")
    nc.tensor.transpose(tb_ps[0:64, 0:64], tsmT[:], idf32[:])
    tsm = big.tile([C, C], DT_OUT, tag="tsm")
    nc.vector.tensor_copy(tsm[:], tb_ps[0:64, 0:64])

    # ---- out^T[d, n-half] = tsm^T-pair @ xmidT ----
    osb = big.tile([C, MH], dt.float32, tag="osb")
    for k in range(4):
        op = mm.tile([C, 512], dt.float32, tag="mmt")
        nc.tensor.matmul(op[:], tsm[:], xmidT[:, ts(k, 512)],
                         start=True, stop=True)
        nc.vector.tensor_copy(osb[:, ts(k, 512)], op[:])
    nc.sync.dma_start(io["outp"], osb[:])

    for p in (dram, acc, mm, slabp, big, const):
        p.release()


def _prep_inputs(x_latter, x, W, b):
    """Build the 8 per-core input maps from full inputs."""
    B = x_latter.shape[0]
    xr = np.ascontiguousarray(x.reshape(B, C, HW))
    xlr = np.ascontiguousarray(x_latter.reshape(B, C, HW))
    wT = {k: np.ascontiguousarray(W[k].T) for k in W}
    id16 = np.ascontiguousarray(
        np.vstack([np.eye(C), np.eye(C)]).astype(np.float16))
    halfid = np.ascontiguousarray((0.5 * np.eye(C)).astype(np.float32))
    idf32 = np.eye(C, dtype=np.float32)
    maps = []
    for core in range(N_CORES):
        bi, h = core // 2, core % 2
        sl = slice(h * MH, (h + 1) * MH)
        maps.append({
            "xin": xr[bi].copy(),
            "xlat": xlr[bi].copy(),
            "xlat_mh": np.ascontiguousarray(xlr[bi][:, sl]),
            "x_mh": np.ascontiguousarray(xr[bi][:, sl]),
            "wlT": wT["low"], "whT": wT["high"], "wvT": wT["value"],
            "weT": wT["e_conv"], "wlatT": wT["latter"], "wmT": wT["mid"],
            "bl": b["low"], "bh": b["high"], "bv": b["value"],
            "blat": b["latter"], "bm": b["mid"],
            "be1": np.ascontiguousarray(b["e_conv"].reshape(1, C)),
            "idf32": idf32, "id16": id16, "halfid": halfid,
        })
    return maps


def run(inputs, trace=False, trace_cores=None):
    if "nc" not in _CACHE:
        _CACHE["nc"] = build()
    nc = _CACHE["nc"]

    names = ["high", "low", "value", "e_conv", "mid", "latter"]
    W = {n: np.asarray(inputs[f"W_{n}"], dtype=np.float32) for n in names}
    b = {n: np.asarray(inputs[f"b_{n}"], dtype=np.float32).reshape(C, 1)
         for n in names}
    x = np.asarray(inputs["x"], dtype=np.float32)
    x_latter = np.asarray(inputs["x_latter"], dtype=np.float32)
    maps = _prep_inputs(x_latter, x, W, b)

    kw = {}
    if trace:
        kw = dict(trace=True,
                  trace_cores=trace_cores or list(range(N_CORES)))
    res = run_bass_kernel_spmd(nc, maps, core_ids=list(range(N_CORES)), **kw)

    B = x_latter.shape[0]
    out = np.empty((B, C, HW), dtype=np.float32)
    for core in range(N_CORES):
        bi, h = core // 2, core % 2
        out[bi][:, h * MH:(h + 1) * MH] = res.results[core]["outp"]
    H = int(np.sqrt(HW))
    return out.reshape(B, C, H, H), res


def kernel(**inputs):
    out, _ = run(inputs, trace=False)
    return out



# revision 5
# speedup vs baseline: 1.0386x; 1.0386x over previous
"""Trainium2 Bass kernel for nn_HOR_16870631539538 (dense_transformer).

Module (per batch item b, C=64 channels, hw=4096 spatial):
  stage 1: p = x_low^T conv outputs attention [hw,hw], softmax over axis n,
           e = p_sm @ v + x_low
  stage 2: t = conv_e(e) @ xl2_sp  (64x64), softmax over c, out = x_mid @ t_sm

Sharding: 8 cores = 4 batch items x 2 halves of the softmax-column dim (m).
Each core computes exp/softmax for its m-half only (the expensive part).
Key algebraic trick: downstream only needs G = e @ xl2_sp  (64x65 incl. the
ones-row for the e_conv bias term), which is linear in the m-partial e, so the
cross-core combine is ONE 16KB AllReduce of G instead of 1MiB of e.

Layouts: conv outputs channel-major [c, n]; attention computed transposed
pT[m, n] so softmax axis n is the free dim (ACT exp with fused accum_out).
The 1/denominator is folded into the small v matrix, never touching the slab.

Dtypes: inputs and conv weights fp16 (PE runs 16-bit moving data at 1
col/cycle vs ~2 for fp32r); exp slab + vs in bf16 (range: |p| can reach ~45,
exp(p) and 1/d need an 8-bit exponent; no max-subtraction needed); G / t path
fp32 (stage-2 softmax is sensitive to absolute errors in t).
"""

import numpy as np

import concourse.bacc as bacc
import concourse.bass as bass
import concourse.mybir as mybir
import concourse.tile as tile
from concourse.bass_utils import run_bass_kernel_spmd

dt = mybir.dt
AF = mybir.ActivationFunctionType
ALU = mybir.AluOpType

N_CORES = 8
C = 64
HW = 4096
MH = HW // 2           # per-core m-half (2048)
NCHUNK = MH // 128     # 16 m-chunks of 128 rows
NB = HW // 512         # 8 n-blocks of 512

DT_IN = dt.float16     # input tensors + conv weights + pT operands
DT_SLAB = dt.bfloat16  # exp slab + vs (bf16: range-safe for exp w/o max)
DT_G = dt.float32      # e_sp / xl2sp operands for the G matmuls
DT_OUT = dt.float16    # tsm / xmidT operands for the output matmul

USE_COLLECTIVE = True
TRUNC = 99   # debug: 1=convs 2=+transposes 3=+mloop 4=+G 99=full

_CACHE = {}


def build():
    nc = bacc.Bacc("TRN2", target_bir_lowering=False, debug=False,
                   num_devices=N_CORES)

    def din(name, shape, dtype=dt.float32):
        return nc.dram_tensor(name, shape, dtype, kind="ExternalInput").ap()

    xin = din("xin", [C, HW], DT_IN)      # x[b] channel-major, fp16
    xlat = din("xlat", [C, HW], DT_IN)    # x_latter[b], fp16
    wlT = din("wlT", [C, C], DT_IN)
    whT = din("whT", [C, C], DT_IN)
    wvT = din("wvT", [C, C], DT_IN)
    weT = din("weT", [C, C])
    wlatT = din("wlatT", [C, C], DT_IN)
    wmT = din("wmT", [C, C], DT_IN)
    bl = din("bl", [C, 1])
    bh = din("bh", [C, 1])
    bv = din("bv", [C, 1])
    blat = din("blat", [C, 1])
    bm = din("bm", [C, 1])
    be1 = din("be1", [1, C])           # e_conv bias as a row
    idf32 = din("idf32", [C, C])       # identity fp32
    outp = nc.dram_tensor("outp", [C, MH], dt.float32,
                          kind="ExternalOutput").ap()

    with tile.TileContext(nc) as tc:
        _body(nc, tc, locals())
    nc.compile()
    return nc


def _body(nc, tc, io):
    ts = bass.ts

    const = tc.alloc_tile_pool(name="const", bufs=1)
    big = tc.alloc_tile_pool(name="big", bufs=1)
    slabp = tc.alloc_tile_pool(name="slabp", bufs=2)
    mm = tc.alloc_tile_pool(name="mm", bufs=2, space="PSUM")
    acc = tc.alloc_tile_pool(name="acc", bufs=1, space="PSUM")
    dram = tc.alloc_tile_pool(name="dram", bufs=1, space="DRAM")

    # ---- load constants (tiny, fast) ----
    def cload(name, shape, dtype=dt.float32):
        t = const.tile(shape, dtype, tag=name)
        nc.sync.dma_start(t[:], io[name])
        return t

    wlT = cload("wlT", [C, C], DT_IN)
    whT = cload("whT", [C, C], DT_IN)
    wvT = cload("wvT", [C, C], DT_IN)
    weT = cload("weT", [C, C])
    wlatT = cload("wlatT", [C, C], DT_IN)
    wmT = cload("wmT", [C, C], DT_IN)
    bl = cload("bl", [C, 1]); bh = cload("bh", [C, 1])
    bv = cload("bv", [C, 1]); blat = cload("blat", [C, 1])
    bm = cload("bm", [C, 1])
    idf32 = cload("idf32", [C, C])
    be_t = const.tile([65, C], dt.float32, tag="be_t")
    nc.sync.dma_start(be_t[64:65, :], io["be1"])

    # ---- inputs: chunked DMA so convs can start on early chunks ----
    # The host rolls xin/xlat columns so every core's OWN m-half is in
    # columns [0, MH) -- keeps the kernel static across cores.
    xin = big.tile([C, HW], DT_IN, tag="xin")
    xlat = big.tile([C, HW], DT_IN, tag="xlat")
    CH = 1024
    # order: xlat m-half (conv high), xin (conv low/value), xlat rest
    for j in range(2):
        nc.sync.dma_start(xlat[:, ts(j, CH)], io["xlat"][:, ts(j, CH)])
    for j in range(4):
        nc.sync.dma_start(xin[:, ts(j, CH)], io["xin"][:, ts(j, CH)])
    for j in range(2, 4):
        nc.sync.dma_start(xlat[:, ts(j, CH)], io["xlat"][:, ts(j, CH)])

    # ---- conv helper: out_sbuf[c, n] = W @ x + b, evicted via DVE ----
    def conv_chunk(dst, wT, src, bias, j, w=CH, accum=None, ji=0):
        pt = mm.tile([C, CH], dt.float32, tag="mmt")
        for k in range(0, w, 512):
            nc.tensor.matmul(pt[:, k:k + min(512, w - k)], wT[:],
                             src[:, j + k:j + k + min(512, w - k)],
                             start=True, stop=True)
        if accum is None:
            nc.vector.tensor_scalar(dst[:, j:j + w], pt[:, 0:w], bias[:], None,
                                    ALU.add)
        else:
            nc.vector.tensor_scalar(dst[:, j:j + w], pt[:, 0:w], bias[:], 0.0,
                                    ALU.add, ALU.add,
                                    accum_out=accum[:, ji:ji + 1])

    xlowT = big.tile([C, HW], DT_IN, tag="xlowT")      # conv_low(x), full n
    xl_hi = big.tile([C, MH], DT_IN, tag="xl_hi")      # conv_high(xlat) m-half
    v_s = big.tile([C, MH], dt.float32, tag="v_s")     # conv_value(x) m-half
    xl2 = big.tile([C, HW], DT_G, tag="xl2")           # conv_latter(xlat) full
    xmidT = big.tile([C, MH], DT_OUT, tag="xmidT")     # conv_mid(x) n-half
    sacc = big.tile([C, 4], dt.float32, tag="sacc")    # xl2 row-sum parts

    # loop-critical convs first: high (xlat m-half), low (xin full),
    # value (xin m-half)
    for j in range(2):
        conv_chunk(xl_hi, whT, xlat, bh, j * CH)
    for j in range(4):
        conv_chunk(xlowT, wlT, xin, bl, j * CH)
    for j in range(2):
        conv_chunk(v_s, wvT, xin, bv, j * CH)

    # ---- transpose v: [64, MH] -> 16 tiles [128, 64] in v_sp ----
    v_sp = big.tile([128, NCHUNK * C], dt.float32, tag="v_sp")
    for g in range(0, NCHUNK, 8):  # batches of 8 transposes -> one psum tile
        pt = mm.tile([128, 512], dt.float32, tag="mmt")
        for q in range(8):
            i = g + q
            nc.tensor.transpose(pt[:, q * 64:(q + 1) * 64],
                                v_s[:, ts(i, 128)], idf32[:])
        nc.vector.tensor_copy(v_sp[:, g * 64:(g + 8) * 64], pt[:])

    # remaining convs (needed only after the m-loop)
    for j in range(4):
        conv_chunk(xl2, wlatT, xlat, blat, j * CH, accum=sacc, ji=j)
    for j in range(2):
        conv_chunk(xmidT, wmT, xin, bm, j * CH)

    # s[d] = sum_n xl2[d, n]  (per-partition scalar)
    s_col = big.tile([C, 1], dt.float32, tag="s_col")
    nc.vector.reduce_sum(s_col[:], sacc[:], axis=mybir.AxisListType.X)

    def finish(src_ap):
        osb = big.tile([C, MH], dt.float32, tag="osb")
        nc.gpsimd.memset(osb[:], 0.0)
        nc.vector.tensor_copy(osb[:, 0:src_ap.shape[1]], src_ap)
        nc.sync.dma_start(io["outp"], osb[:])
        for p in (dram, acc, mm, slabp, big, const):
            p.release()

    if TRUNC == 1:
        return finish(xlowT[:, 0:MH])

    # ---- transpose xl2: [64, HW] -> 32 tiles [128, 64] in xl2sp ----
    xl2sp = big.tile([128, 32 * C], DT_G, tag="xl2sp")
    for g in range(0, 32, 8):
        pt = mm.tile([128, 512], DT_G, tag="mmt")
        for q in range(8):
            i = g + q
            nc.tensor.transpose(pt[:, q * 64:(q + 1) * 64],
                                xl2[:, ts(i, 128)], idf32[:])
        nc.vector.tensor_copy(xl2sp[:, g * 64:(g + 8) * 64], pt[:])

    if TRUNC == 2:
        return finish(xl2sp[0:64, 0:2048])

    # ---- eT accumulators: 4 psum tiles [128, 512]; partition half p holds
    # n-block 2k+p. ----
    accs = [acc.tile([128, 512], dt.float32, tag=f"acc{k}", name=f"acc{k}")
            for k in range(4)]

    # x_low residual in acc layout: partitions 0-63 = even n-blocks,
    # 64-127 = odd n-blocks (partition-moving SBUF DMA)
    xlow_acc = big.tile([128, 4 * 512], DT_IN, tag="xlow_acc")
    for k in range(4):
        nc.sync.dma_start(xlow_acc[0:64, ts(k, 512)],
                          xlowT[:, ts(2 * k, 512)])
        nc.sync.dma_start(xlow_acc[64:128, ts(k, 512)],
                          xlowT[:, ts(2 * k + 1, 512)])

    # ---- stage-1 m-loop ----
    for i in range(NCHUNK):
        slab = slabp.tile([128, HW], DT_SLAB, tag="slab")
        dacc = slabp.tile([128, 4], dt.float32, tag="dacc")
        for j in range(4):  # pT sub-passes of [128, 1024]
            pt = mm.tile([128, 1024], dt.float32, tag="mmt")
            for k in range(2):
                nc.tensor.matmul(pt[:, k * 512:(k + 1) * 512],
                                 xl_hi[:, ts(i, 128)],
                                 xlowT[:, j * 1024 + k * 512:
                                       j * 1024 + (k + 1) * 512],
                                 start=True, stop=True)
            nc.scalar.activation(slab[:, j * 1024:(j + 1) * 1024], pt[:],
                                 AF.Exp, accum_out=dacc[:, j:j + 1])
        dsum = slabp.tile([128, 1], dt.float32, tag="dsum")
        nc.vector.reduce_sum(dsum[:], dacc[:], axis=mybir.AxisListType.X)
        rec = slabp.tile([128, 1], dt.float32, tag="rec")
        nc.vector.reciprocal(rec[:], dsum[:])
        vs = slabp.tile([128, C], DT_SLAB, tag="vs")
        nc.vector.tensor_scalar(vs[:], v_sp[:, ts(i, C)], rec[:], None,
                                ALU.mult)
        first = i == 0
        last = i == NCHUNK - 1
        for k in range(4):
            for p in range(2):
                blk = 2 * k + p
                nc.tensor.matmul(accs[k][p * 64:(p + 1) * 64, :], vs[:],
                                 slab[:, ts(blk, 512)], start=first,
                                 stop=last, skip_group_check=True)

    # ---- evict e + 0.5*x_low residual (block-permuted [128, 2048]) ----
    # (0.5: the AllReduce over the core pair sums two copies of the residual
    # and bias-row terms, so each core contributes half.)
    e_h = big.tile([128, 4 * 512], DT_G, tag="e_h")
    for k in range(4):
        nc.vector.scalar_tensor_tensor(e_h[:, ts(k, 512)],
                                       xlow_acc[:, ts(k, 512)], 0.5,
                                       accs[k][:], ALU.mult, ALU.add)

    if TRUNC == 3:
        return finish(e_h[0:64, :])

    # move odd-block rows (partitions 64-127) down to 0-63 via SBUF DMA
    e_h2 = big.tile([C, 4 * 512], DT_G, tag="e_h2")
    nc.sync.dma_start(e_h2[:], e_h[64:128, :])

    # ---- transpose e -> e_sp tiles [128, 64] in n-tile order ----
    e_sp = big.tile([128, 32 * C], DT_G, tag="e_sp")
    for g in range(0, 32, 8):
        pt = mm.tile([128, 512], DT_G, tag="mmt")
        for q in range(8):
            t_idx = g + q              # n-tile index (n = t_idx*128)
            blk = t_idx // 4           # n-block
            sl = t_idx % 4             # 128-col slice within block
            k = blk // 2
            p = blk % 2
            src = e_h if p == 0 else e_h2
            nc.tensor.transpose(
                pt[:, q * 64:(q + 1) * 64],
                src[0:64, k * 512 + sl * 128:k * 512 + (sl + 1) * 128],
                idf32[:])
        nc.vector.tensor_copy(e_sp[:, g * 64:(g + 8) * 64], pt[:])

    if TRUNC == 35:
        return finish(e_sp[0:64, 0:2048])

    # ---- G = sum_n e_sp^T-pair xl2sp  -> [64(a), 64(d)] psum ----
    gps = acc.tile([128, 512], dt.float32, tag="acc0")
    G = gps[0:64, 0:64]
    for t_idx in range(32):
        nc.tensor.matmul(G, e_sp[:, ts(t_idx, C)], xl2sp[:, ts(t_idx, C)],
                         start=(t_idx == 0), stop=(t_idx == 31),
                         skip_group_check=True)

    if TRUNC == 36:
        gtmp = big.tile([C, C], dt.float32, tag="gtmp")
        nc.vector.tensor_copy(gtmp[:], G)
        return finish(gtmp[:])

    # transpose s_col [64,1] -> s row [1, 64], scaled 0.5 at eviction
    spt = mm.tile([128, 512], dt.float32, tag="mmt")
    nc.tensor.transpose(spt[0:1, 0:64], s_col[:], idf32[:])

    gs_stage = big.tile([65, C], dt.float32, tag="gs_stage")
    nc.vector.tensor_copy(gs_stage[0:64, :], G)
    nc.vector.tensor_scalar(gs_stage[64:65, :], spt[0:1, 0:64], 0.5, None,
                            ALU.mult)

    if TRUNC == 4:
        return finish(gs_stage[0:64, :])

    # ---- AllReduce G over core pairs ----
    gs_red = big.tile([65, C], dt.float32, tag="gs_red")
    if USE_COLLECTIVE:
        g_in = dram.tile([65, C], dt.float32, tag="g_in")
        g_out = dram.tile([65, C], dt.float32, tag="g_out")
        nc.sync.dma_start(g_in[:], gs_stage[:])
        nc.gpsimd.collective_compute(
            "AllReduce", ALU.add,
            ins=[g_in.opt()], outs=[g_out.opt()],
            replica_groups=[[0, 1], [2, 3], [4, 5], [6, 7]],
        )
        nc.sync.dma_start(gs_red[:], g_out[:])
    else:
        nc.vector.tensor_copy(gs_red[:], gs_stage[:])

    # ---- t = We @ G + be x s  (fp32 matmuls) ----
    tps = mm.tile([128, 512], dt.float32, tag="mmt")
    t_ps = tps[0:64, 0:64]
    nc.tensor.matmul(t_ps, weT[:], gs_red[0:64, :], start=True, stop=False,
                     skip_group_check=True)
    nc.tensor.matmul(t_ps, be_t[64:65, :], gs_red[64:65, :], start=False,
                     stop=True, tile_position=(64, 0), skip_group_check=True)
    t_s = big.tile([C, C], dt.float32, tag="t_s")
    nc.vector.tensor_copy(t_s[:], t_ps)

    # ---- softmax over c: transpose -> [d, c], exp w/ max, normalize ----
    tt_ps = mm.tile([128, 512], dt.float32, tag="mmt")
    nc.tensor.transpose(tt_ps[0:64, 0:64], t_s[:], idf32[:])
    tmax = big.tile([C, 1], dt.float32, tag="tmax")
    nc.vector.reduce_max(tmax[:], tt_ps[0:64, 0:64], axis=mybir.AxisListType.X)
    nmax = big.tile([C, 1], dt.float32, tag="nmax")
    nc.vector.tensor_scalar(nmax[:], tmax[:], -1.0, None, ALU.mult)
    texp = big.tile([C, C], dt.float32, tag="texp")
    tsum = big.tile([C, 1], dt.float32, tag="tsum")
    nc.scalar.activation(texp[:], tt_ps[0:64, 0:64], AF.Exp, bias=nmax[:],
                         accum_out=tsum[:])
    trec = big.tile([C, 1], dt.float32, tag="trec")
    nc.vector.reciprocal(trec[:], tsum[:])
    tsmT = big.tile([C, C], dt.float32, tag="tsmT")
    nc.vector.tensor_scalar(tsmT[:], texp[:], trec[:], None, ALU.mult)

    # transpose back -> tsm[c, d] fp16 for the output matmul
    tb_ps = mm.tile([128, 512], dt.float32, tag="mmt")
    nc.tensor.transpose(tb_ps[0:64, 0:64], tsmT[:], idf32[:])
    tsm = big.tile([C, C], DT_OUT, tag="tsm")
    nc.vector.tensor_copy(tsm[:], tb_ps[0:64, 0:64])

    # ---- out^T[d, n-half] = tsm^T-pair @ xmidT ----
    osb = big.tile([C, MH], dt.float32, tag="osb")
    for k in range(4):
        op = mm.tile([C, 512], dt.float32, tag="mmt")
        nc.tensor.matmul(op[:], tsm[:], xmidT[:, ts(k, 512)],
                         start=True, stop=True)
        nc.vector.tensor_copy(osb[:, ts(k, 512)], op[:])
    nc.sync.dma_start(io["outp"], osb[:])

    for p in (dram, acc, mm, slabp, big, const):
        p.release()


def _prep_inputs(x_latter, x, W, b):
    """Build the 8 per-core input maps from full inputs."""
    B = x_latter.shape[0]
    xr = x.reshape(B, C, HW).astype(np.float16)
    xlr = x_latter.reshape(B, C, HW).astype(np.float16)
    wT = {k: np.ascontiguousarray(W[k].T) for k in W}
    wT16 = {k: wT[k].astype(np.float16) for k in wT}
    idf32 = np.eye(C, dtype=np.float32)
    maps = []
    for core in range(N_CORES):
        bi, h = core // 2, core % 2
        # roll columns so this core's own m-half sits at columns [0, MH)
        xin_c = np.roll(xr[bi], -h * MH, axis=1) if h else xr[bi]
        xlat_c = np.roll(xlr[bi], -h * MH, axis=1) if h else xlr[bi]
        maps.append({
            "xin": np.ascontiguousarray(xin_c),
            "xlat": np.ascontiguousarray(xlat_c),
            "wlT": wT16["low"], "whT": wT16["high"], "wvT": wT16["value"],
            "weT": wT["e_conv"], "wlatT": wT16["latter"], "wmT": wT16["mid"],
            "bl": b["low"], "bh": b["high"], "bv": b["value"],
            "blat": b["latter"], "bm": b["mid"],
            "be1": np.ascontiguousarray(b["e_conv"].reshape(1, C)),
            "idf32": idf32,
            "hhalf": np.array([[h]], dtype=np.int32),
        })
    return maps


def run(inputs, trace=False, trace_cores=None):
    if "nc" not in _CACHE:
        _CACHE["nc"] = build()
    nc = _CACHE["nc"]

    names = ["high", "low", "value", "e_conv", "mid", "latter"]
    W = {n: np.asarray(inputs[f"W_{n}"], dtype=np.float32) for n in names}
    b = {n: np.asarray(inputs[f"b_{n}"], dtype=np.float32).reshape(C, 1)
         for n in names}
    x = np.asarray(inputs["x"], dtype=np.float32)
    x_latter = np.asarray(inputs["x_latter"], dtype=np.float32)
    maps = _prep_inputs(x_latter, x, W, b)

    kw = {}
    if trace:
        kw = dict(trace=True,
                  trace_cores=trace_cores or list(range(N_CORES)))
    res = run_bass_kernel_spmd(nc, maps, core_ids=list(range(N_CORES)), **kw)

    B = x_latter.shape[0]
    out = np.empty((B, C, HW), dtype=np.float32)
    for core in range(N_CORES):
        bi, h = core // 2, core % 2
        out[bi][:, h * MH:(h + 1) * MH] = res.results[core]["outp"]
    H = int(np.sqrt(HW))
    return out.reshape(B, C, H, H), res


def kernel(**inputs):
    out, _ = run(inputs, trace=False)
    return out
